# revision 15
# baseline (speedup 1.0000x reference)
"""Trainium2 Bass kernel for nn_EuclideanLoss.

Math (matches the oracle):
    y_t  = transpose(y, (0, 2, 1))                 # [B, N, D]
    pd   = sqrt(sum((x - y_t)^2, axis=-1))         # [B, N]
    dist = mean(pd, axis=0); dist[1:3] *= 1.5
    loss = mean(dist)

Strategy: data-parallel over batch — each of the 8 NeuronCores takes 4
batches and computes its pair distances pd[b, n] on device; the tiny [B, N]
result is gathered to the host, which finishes mean/scale/mean in float64.

The problem is DMA-bound (16MB of fp32 input per core, ~44us at the
measured ~390GB/s single-core HBM rate); everything else must hide under
the stream.  Measured engine facts driving the design (see traces):
  * DVE fp32 ops are 1x mode, (N+151)/0.96 ns — sub on [128,1024] is
    1.21us and tensor_reduce has NO fast DVE mode at any dtype.  DVE alone
    (38.7us) was the old critical path, so reduces alternate between DVE
    and the otherwise-idle GpSimd (Pool) engine.
  * bf16 was tried and reverted: PE transposes run ~2x SLOWER in bf16
    than fp32, and the SWDGE cast-load path throttles the stream.
  * Loads stay on the two HWDGE queues: x via nc.sync, y via nc.scalar —
    two logical queues keep more descriptors in flight per SDMA engine.
Layouts (address-sequential HBM descriptors):
  * y[b] ([64, 8192] row-major) loads FLAT into [128, 4096]: partition
    p = 2d + nh holds y[d, nh*4096 : (nh+1)*4096] — 16KB descriptors.
    y3 loads early so batch-3's PE transposes are off the tail path.
  * x[b] loads as [128, 2, 32, 64] = (q, nh, c, d), n = nh*4096+q*32+c,
    in 2 pieces (16 c-cols, 4KB descriptors); the LAST batch in 4 pieces
    (16,8,4,4) so the final sub->square->reduce chain is short.
Compute per batch (c-chunks of 8; last batch 8,8,8,4,4):
  PE   transposes y_v[:, c, :] ([128, 128]) -> PSUM yT[q, c, 2d+nh]
  DVE  diff = x - yT
  ACT  sq = Square(diff)
  DVE/GPS (alternating) reduce over d -> d2 fp32
  ACT  per-batch pd = Sqrt(d2) (hidden under the stream; LUT pre-warmed),
       one contiguous fp32 store at the end.

Output o[b, p, g, nh, c] = pd[b, nh*4096 + p*32 + g*8 + c]; host undoes it.
"""

import numpy as np

import concourse.bacc as bacc
import concourse.bass as bass
import concourse.mybir as mybir
import concourse.tile as tile
from concourse import masks
from concourse.bass_utils import run_bass_kernel_spmd

# Cap the semaphore universe walrus manages (this kernel declares sems
# 150-166 only).  Measured on the fp32 baseline build: -6..7us — the
# compiled schedule loses the end-of-stream DMA straggle that left DVE
# with a post-stream backlog.
import concourse.bass_utils as _bu

_orig_run_command = _bu.run_command

def _run_command_semcap(cmd, **kw):
    if isinstance(cmd, list) and cmd and "walrus_driver" in str(cmd[0]):
        cmd = list(cmd) + ["--max-sem-num=176"]
    return _orig_run_command(cmd, **kw)

_bu.run_command = _run_command_semcap

B, N, D = 32, 8192, 64
NCORES = 8
BL = B // NCORES        # 4 local batches per core
P = 128                 # SBUF partitions
NH = 2                  # n-halves per batch (partition interleave of y)
CPB = N // NH // P      # 32 consecutive x rows per partition per half
NG = 4                  # c-groups per batch (transpose granularity)
GC = CPB // NG          # 8 columns per group

F32 = mybir.dt.float32
BF16 = mybir.dt.bfloat16

# compute chunks (c-column ranges) per batch; last batch tapers so the
# final chain operates on little data
CHUNKS = [(c, c + GC) for c in range(0, CPB, GC)]
CHUNKS_LAST = [(0, 8), (8, 16), (16, 24), (24, 28), (28, 32)]
# DMA pieces for x (subtile deps connect compute chunks to pieces)
XPIECES = [(0, 8), (8, 16), (16, 32)]
XPIECES_LAST = [(0, 8), (8, 16), (16, 24), (24, 28), (28, 32)]


def _build() -> bass.Bass:
    # Bacc (not plain Bass): its compile() pass splits sem waits across
    # event-semaphore instructions — TRN2 instructions hold at most one wait,
    # and this walrus build rejects multi-wait instructions outright.
    nc = bacc.Bacc("TRN2", target_bir_lowering=False, debug=False, num_devices=NCORES)
    x_d = nc.dram_tensor("x", [BL, N, D], F32, kind="ExternalInput")
    y_d = nc.dram_tensor("y", [BL, D, N], F32, kind="ExternalInput")
    o_d = nc.dram_tensor("o", [P, BL, NG, NH, GC], F32, kind="ExternalOutput")
    nc.dram_tensor("cachebust_v6semcap", [1, 1], F32, kind="Internal")

    with tile.TileContext(nc) as tc:
        with (
            tc.tile_pool(name="const", bufs=1) as cpool,
            tc.tile_pool(name="io", bufs=4) as iopool,
            tc.tile_pool(name="work", bufs=4) as wpool,
            tc.tile_pool(name="psum", bufs=4, space="PSUM") as ppool,
        ):
            # ---- constants FIRST (GpSimd also issues the SWDGE loads) ------------------------
            ident = cpool.tile([P, P], BF16)
            masks.make_identity(nc, ident[:])
            d2a = cpool.tile([P, BL, NG, NH, GC], F32)
            pda = cpool.tile([P, BL, NG, NH, GC], F32)
            # Warm the Sqrt LUT during the DMA fill so the per-batch sqrts
            # do not stall ~1.3us on a lazy ACT_TABLE_LOAD.
            warm = cpool.tile([P, 1], F32)
            nc.scalar.activation(
                warm[:], ident[:, 0:1], mybir.ActivationFunctionType.Sqrt
            )

            # ---- issue every input DMA up front ------------------------
            x_tiles, y_tiles = [], []
            for b in range(BL):
                x_tiles.append(
                    iopool.tile([P, NH, CPB, D], BF16, tag="x", name=f"x{b}")
                )
                y_tiles.append(
                    iopool.tile([P, NH * CPB * D], BF16, tag="y", name=f"y{b}")
                )

            def load_y(b):
                # SWDGE cast fp32->bf16 during the DMA; partition nh*64+d
                # holds y[d, nh*4096:(nh+1)*4096] (one DMA per half: the
                # (nh d) grouping isn't expressible in one rearrange)
                for nh in range(NH):
                    nc.gpsimd.dma_start(
                        y_tiles[b][nh * 64 : (nh + 1) * 64, :],
                        y_d[b][:, nh * (N // NH) : (nh + 1) * (N // NH)],
                    )

            def load_x(b, c0, c1):
                xsrc = x_d[b].rearrange("(nh q c) d -> q nh c d", nh=NH, c=CPB)
                nc.gpsimd.dma_start(
                    x_tiles[b][:, :, c0:c1, :], xsrc[:, :, c0:c1, :]
                )

            load_y(0)
            for c0, c1 in XPIECES:
                load_x(0, c0, c1)
            load_y(1)
            load_y(3)            # early: batch-3 transposes off the tail path
            for c0, c1 in XPIECES:
                load_x(1, c0, c1)
            load_y(2)
            for c0, c1 in XPIECES:
                load_x(2, c0, c1)
            for c0, c1 in XPIECES_LAST:
                load_x(3, c0, c1)

            # ---- per-batch compute, software-pipelined ------------------
            # DVE is in-order; emitting sub(i) -> reduce(i) back-to-back
            # makes DVE idle ~1.1us per chunk waiting for ACT's square.
            # Instead emit sub(i+1) BEFORE reduce(i): DVE computes the next
            # sub while ACT squares chunk i, so reduce(i)'s input is ready
            # the moment DVE reaches it.
            def emit_transposes(b):
                y_v = y_tiles[b].rearrange("p (q c) -> p c q", c=CPB)
                for g in range(NG):
                    t = ppool.tile([P, GC, P], BF16, tag="yT", name=f"yT{b}_{g}")
                    for c in range(GC):
                        nc.tensor.transpose(
                            t[:, c, :], y_v[:, g * GC + c, :], ident[:]
                        )
                    yT[(b, g)] = t

            def emit_sub_sq(b, c0, c1):
                g, gc0 = c0 // GC, c0 % GC
                w = c1 - c0
                diff = wpool.tile([P, NH, w, D], BF16, tag="diff", name=f"df{b}{c0}")
                nc.vector.tensor_sub(
                    diff[:],
                    x_tiles[b][:, :, c0:c1, :],
                    yT[(b, g)][:, gc0 : gc0 + w, :].rearrange(
                        "p c (nh d) -> p nh c d", nh=NH
                    ),
                )
                sq = wpool.tile([P, NH, w, D], BF16, tag="sq", name=f"sq{b}{c0}")
                nc.scalar.activation(
                    sq[:], diff[:], mybir.ActivationFunctionType.Square
                )
                return sq

            def emit_reduce(b, c0, c1, sq):
                g, gc0 = c0 // GC, c0 % GC
                w = c1 - c0
                nc.vector.tensor_reduce(
                    d2a[:, b, g, :, gc0 : gc0 + w],
                    sq[:],
                    axis=mybir.AxisListType.X,
                    op=mybir.AluOpType.add,
                )
                if c1 == CPB:
                    # per-batch sqrt: batches 0-2 hide under the stream;
                    # only batch 3's short [P, 64] sqrt sits in the tail
                    nc.scalar.activation(
                        pda[:, b], d2a[:, b], mybir.ActivationFunctionType.Sqrt
                    )

            yT = {}
            all_chunks = []
            for b in range(BL):
                for c0, c1 in (CHUNKS_LAST if b == BL - 1 else CHUNKS):
                    all_chunks.append((b, c0, c1))

            pending = None  # (b, c0, c1, sq) awaiting its reduce
            for b, c0, c1 in all_chunks:
                if c0 == 0:
                    emit_transposes(b)
                sq = emit_sub_sq(b, c0, c1)
                if pending is not None:
                    emit_reduce(*pending)
                pending = (b, c0, c1, sq)
            emit_reduce(*pending)

            # One contiguous store for all batches: per-batch strided stores
            # interleave small descriptors into the input stream.
            nc.sync.dma_start(o_d[:], pda[:])
    nc.finalize()
    return nc


_NC_CACHE: list = []


def _get_program() -> bass.Bass:
    if not _NC_CACHE:
        _NC_CACHE.append(_build())
    return _NC_CACHE[0]


def kernel(x: np.ndarray, y: np.ndarray) -> np.ndarray:
    x = np.ascontiguousarray(np.asarray(x, dtype=np.float32))
    y = np.ascontiguousarray(np.asarray(y, dtype=np.float32))
    assert x.shape == (B, N, D) and y.shape == (B, D, N)

    nc = _get_program()
    in_maps = [
        {"x": x[i * BL : (i + 1) * BL], "y": y[i * BL : (i + 1) * BL]}
        for i in range(NCORES)
    ]
    res = run_bass_kernel_spmd(nc, in_maps, list(range(NCORES)))
    o = np.stack([res.results[i]["o"] for i in range(NCORES)])  # [8, P, BL, NG, NH, GC]
    # o[core, p, b, g, nh, c] = pd[core*BL + b, nh*4096 + p*32 + g*8 + c]
    pd = (
        o.transpose(0, 2, 4, 1, 3, 5)  # (core, b, nh, p, g, c)
        .reshape(B, N)
    )

    dist = pd.mean(axis=0, dtype=np.float64)
    dist[1:3] *= 1.5
    return np.asarray(dist.mean(), dtype=np.float32)


# revision 16
# speedup vs baseline: 1.0144x; 1.0144x over previous
"""Trainium2 Bass kernel for nn_EuclideanLoss.

Math (matches the oracle):
    y_t  = transpose(y, (0, 2, 1))                 # [B, N, D]
    pd   = sqrt(sum((x - y_t)^2, axis=-1))         # [B, N]
    dist = mean(pd, axis=0); dist[1:3] *= 1.5
    loss = mean(dist)

Strategy: data-parallel over batch — each of the 8 NeuronCores takes 4
batches and computes its pair distances pd[b, n] on device; the tiny [B, N]
result is gathered to the host, which finishes mean/scale/mean in float64.

The problem is DMA-bound (16MB of input per core), so both loads are laid
out to produce fully address-sequential HBM descriptors (~370 GB/s measured;
the naive row-strided y load runs at ~200 GB/s and 256B-descriptor x loads
at ~60 GB/s):
  * y[b] ([64, 8192] row-major) loads FLAT into [128, 4096]: partition
    p = 2d + nh holds y[d, nh*4096 : (nh+1)*4096] — pure 16KB-contiguous
    descriptors.  (nh = which half of the batch's n-range)
  * x[b] loads as [128, 2, 32, 64] = (q, nh, c, d) with
    n = nh*4096 + q*32 + c — 32 consecutive rows = 8KB descriptors.
Compute per batch (c-groups of 8 columns):
  PE   32 transposes y_v[:, c, :] ([128, 128]) -> PSUM yT[q, c, 2d+nh],
       aligning y to x's n-to-partition map.
  DVE  diff = x - yT  (yT read through a stride-permuted PSUM view)
  ACT  sq = Square(diff)
  DVE  reduce over d -> d2[p, g, nh, c]
  ACT  pd = Sqrt(d2); DMA out.

Output o[b, p, g, nh, c] = pd[b, nh*4096 + p*32 + g*8 + c]; host undoes it.
"""

import numpy as np

import concourse.bacc as bacc
import concourse.bass as bass
import concourse.mybir as mybir
import concourse.tile as tile
from concourse import masks
from concourse.bass_utils import run_bass_kernel_spmd

# Cap the semaphore universe walrus manages (this kernel declares sems
# 150-166 only).  Measured effect: -6..7us vs the unflagged build — the
# compiled schedule loses the end-of-stream DMA straggle that used to leave
# DVE with an ~8us post-stream backlog (median 63.8us vs 69.8-71.0 across
# 6 unflagged runs; the per-sem teardown clears themselves are unchanged).
import concourse.bass_utils as _bu

_orig_run_command = _bu.run_command

def _run_command_semcap(cmd, **kw):
    if isinstance(cmd, list) and cmd and "walrus_driver" in str(cmd[0]):
        cmd = list(cmd) + ["--max-sem-num=176"]
    return _orig_run_command(cmd, **kw)

_bu.run_command = _run_command_semcap

B, N, D = 32, 8192, 64
NCORES = 8
BL = B // NCORES        # 4 local batches per core
P = 128                 # SBUF partitions
NH = 2                  # n-halves per batch (partition interleave of y)
CPB = N // NH // P      # 32 consecutive x rows per partition per half
NG = 4                  # c-groups per batch
GC = CPB // NG          # 8 columns per group

F32 = mybir.dt.float32


def _build() -> bass.Bass:
    # Bacc (not plain Bass): its compile() pass splits sem waits across
    # event-semaphore instructions — TRN2 instructions hold at most one wait,
    # and this walrus build rejects multi-wait instructions outright.
    nc = bacc.Bacc("TRN2", target_bir_lowering=False, debug=False, num_devices=NCORES)
    x_d = nc.dram_tensor("x", [BL, N, D], F32, kind="ExternalInput")
    y_d = nc.dram_tensor("y", [BL, D, N], F32, kind="ExternalInput")
    o_d = nc.dram_tensor("o", [P, BL, NG, NH, GC], F32, kind="ExternalOutput")
    nc.dram_tensor("cachebust_semcap176", [1, 1], F32, kind="Internal")

    with tile.TileContext(nc) as tc:
        with (
            tc.tile_pool(name="const", bufs=1) as cpool,
            tc.tile_pool(name="io", bufs=4) as iopool,
            tc.tile_pool(name="work", bufs=4) as wpool,
            tc.tile_pool(name="res", bufs=2) as rpool,
            tc.tile_pool(name="psum", bufs=4, space="PSUM") as ppool,
        ):
            ident = cpool.tile([P, P], F32)
            masks.make_identity(nc, ident[:])
            d2a = cpool.tile([P, BL, NG, NH, GC], F32)
            # Warm the Sqrt LUT during the DMA fill so the final sqrt does
            # not stall ~1.3us on a lazy ACT_TABLE_LOAD.
            warm = cpool.tile([P, 1], F32)
            nc.scalar.activation(
                warm[:], ident[:, 0:1], mybir.ActivationFunctionType.Sqrt
            )

            for b in range(BL):
                x_t = iopool.tile([P, NH, CPB, D], F32, tag="x")
                y_t = iopool.tile([P, NH * CPB * D], F32, tag="y")
                # y first: the transposes depend only on y, so PE can start
                # while x is still streaming in.  x splits into per-group
                # c-range DMAs so subs can begin before the whole batch lands.
                nc.sync.dma_start(
                    y_t[:], y_d[b].rearrange("d (nh n) -> (d nh) n", nh=NH)
                )
                xsrc = x_d[b].rearrange("(nh q c) d -> q nh c d", nh=NH, c=CPB)
                for g in range(NG):
                    nc.sync.dma_start(
                        x_t[:, :, g * GC : (g + 1) * GC, :],
                        xsrc[:, :, g * GC : (g + 1) * GC, :],
                    )

                # column q of slice c holds n-offset q*32+c within each half
                y_v = y_t[:].rearrange("p (q c) -> p c q", c=CPB)
                for g in range(NG):
                    yT = ppool.tile([P, GC, P], F32, tag="yT")
                    for c in range(GC):
                        nc.tensor.transpose(
                            yT[:, c, :], y_v[:, g * GC + c, :], ident[:]
                        )

                    diff = wpool.tile([P, NH, GC, D], F32, tag="diff")
                    nc.vector.tensor_sub(
                        diff[:],
                        x_t[:, :, g * GC : (g + 1) * GC, :],
                        yT[:].rearrange("p c (d nh) -> p nh c d", nh=NH),
                    )
                    sq = wpool.tile([P, NH, GC, D], F32, tag="sq")
                    nc.scalar.activation(
                        sq[:], diff[:], mybir.ActivationFunctionType.Square
                    )
                    nc.vector.tensor_reduce(
                        d2a[:, b, g, :, :],
                        sq[:],
                        axis=mybir.AxisListType.X,
                        op=mybir.AluOpType.add,
                    )

            # One Sqrt + one contiguous store for all batches: per-batch
            # strided stores measured ~13us slower (they interleave small
            # descriptors into the input stream).
            pda = rpool.tile([P, BL, NG, NH, GC], F32, tag="pd")
            nc.scalar.activation(pda[:], d2a[:], mybir.ActivationFunctionType.Sqrt)
            nc.sync.dma_start(o_d[:], pda[:])
    nc.finalize()
    return nc


_NC_CACHE: list = []


def _get_program() -> bass.Bass:
    if not _NC_CACHE:
        _NC_CACHE.append(_build())
    return _NC_CACHE[0]


def kernel(x: np.ndarray, y: np.ndarray) -> np.ndarray:
    x = np.ascontiguousarray(np.asarray(x, dtype=np.float32))
    y = np.ascontiguousarray(np.asarray(y, dtype=np.float32))
    assert x.shape == (B, N, D) and y.shape == (B, D, N)

    nc = _get_program()
    in_maps = [
        {"x": x[i * BL : (i + 1) * BL], "y": y[i * BL : (i + 1) * BL]}
        for i in range(NCORES)
    ]
    res = run_bass_kernel_spmd(nc, in_maps, list(range(NCORES)))
    o = np.stack([res.results[i]["o"] for i in range(NCORES)])  # [8, P, BL, NG, NH, GC]
    # o[core, p, b, g, nh, c] = pd[core*BL + b, nh*4096 + p*32 + g*8 + c]
    pd = (
        o.transpose(0, 2, 4, 1, 3, 5)  # (core, b, nh, p, g, c)
        .reshape(B, N)
    )

    dist = pd.mean(axis=0, dtype=np.float64)
    dist[1:3] *= 1.5
    return np.asarray(dist.mean(), dtype=np.float32)


# revision 17
# speedup vs baseline: 1.1219x; 1.1060x over previous
"""Trainium2 Bass kernel for nn_EuclideanLoss.

Math (matches the oracle):
    y_t  = transpose(y, (0, 2, 1))                 # [B, N, D]
    pd   = sqrt(sum((x - y_t)^2, axis=-1))         # [B, N]
    dist = mean(pd, axis=0); dist[1:3] *= 1.5
    loss = mean(dist)

Strategy: data-parallel over batch — each of the 8 NeuronCores takes 4
batches and computes its pair distances pd[b, n] on device; the tiny [B, N]
result is gathered to the host, which finishes mean/scale/mean in float64.

The problem is DMA-bound (16MB of input per core), so both loads are laid
out to produce fully address-sequential HBM descriptors (~370 GB/s measured;
the naive row-strided y load runs at ~200 GB/s and 256B-descriptor x loads
at ~60 GB/s):
  * y[b] ([64, 8192] row-major) loads FLAT into [128, 4096]: partition
    p = 2d + nh holds y[d, nh*4096 : (nh+1)*4096] — pure 16KB-contiguous
    descriptors.  (nh = which half of the batch's n-range)
  * x[b] loads as [128, 2, 32, 64] = (q, nh, c, d) with
    n = nh*4096 + q*32 + c — 32 consecutive rows = 8KB descriptors.
Compute per batch (c-groups of 8 columns):
  PE   32 transposes y_v[:, c, :] ([128, 128]) -> PSUM yT[q, c, 2d+nh],
       aligning y to x's n-to-partition map.
  DVE  diff = x - yT  (yT read through a stride-permuted PSUM view)
  ACT  sq = Square(diff)
  DVE  reduce over d -> d2[p, g, nh, c]
  ACT  pd = Sqrt(d2); DMA out.

Output o[b, p, g, nh, c] = pd[b, nh*4096 + p*32 + g*8 + c]; host undoes it.
"""

import numpy as np

import concourse.bacc as bacc
import concourse.bass as bass
import concourse.mybir as mybir
import concourse.tile as tile
from concourse import masks
from concourse.bass_utils import run_bass_kernel_spmd

# Cap the semaphore universe walrus manages (this kernel declares sems
# 150-166 only).  Measured effect: -6..7us vs the unflagged build — the
# compiled schedule loses the end-of-stream DMA straggle that used to leave
# DVE with an ~8us post-stream backlog (median 63.8us vs 69.8-71.0 across
# 6 unflagged runs; the per-sem teardown clears themselves are unchanged).
import concourse.bass_utils as _bu

_orig_run_command = _bu.run_command

def _run_command_semcap(cmd, **kw):
    if isinstance(cmd, list) and cmd and "walrus_driver" in str(cmd[0]):
        cmd = list(cmd) + ["--max-sem-num=176"]
    return _orig_run_command(cmd, **kw)

_bu.run_command = _run_command_semcap

B, N, D = 32, 8192, 64
NCORES = 8
BL = B // NCORES        # 4 local batches per core
P = 128                 # SBUF partitions
NH = 2                  # n-halves per batch (partition interleave of y)
CPB = N // NH // P      # 32 consecutive x rows per partition per half
NG = 4                  # c-groups per batch
GC = CPB // NG          # 8 columns per group

F32 = mybir.dt.float32


def _build() -> bass.Bass:
    # Bacc (not plain Bass): its compile() pass splits sem waits across
    # event-semaphore instructions — TRN2 instructions hold at most one wait,
    # and this walrus build rejects multi-wait instructions outright.
    nc = bacc.Bacc("TRN2", target_bir_lowering=False, debug=False, num_devices=NCORES)
    x_d = nc.dram_tensor("x", [BL, N, D], F32, kind="ExternalInput")
    y_d = nc.dram_tensor("y", [BL, D, N], F32, kind="ExternalInput")
    o_d = nc.dram_tensor("o", [P, BL, NG, NH, GC], F32, kind="ExternalOutput")
    nc.dram_tensor("cachebust_semcap176_pipelined", [1, 1], F32, kind="Internal")

    with tile.TileContext(nc) as tc:
        with (
            tc.tile_pool(name="const", bufs=1) as cpool,
            tc.tile_pool(name="io", bufs=4) as iopool,
            tc.tile_pool(name="work", bufs=4) as wpool,
            tc.tile_pool(name="res", bufs=2) as rpool,
            tc.tile_pool(name="psum", bufs=4, space="PSUM") as ppool,
        ):
            ident = cpool.tile([P, P], F32)
            masks.make_identity(nc, ident[:])
            d2a = cpool.tile([P, BL, NG, NH, GC], F32)
            # Warm the Sqrt LUT during the DMA fill so the final sqrt does
            # not stall ~1.3us on a lazy ACT_TABLE_LOAD.
            warm = cpool.tile([P, 1], F32)
            nc.scalar.activation(
                warm[:], ident[:, 0:1], mybir.ActivationFunctionType.Sqrt
            )

            # DMA structure kept byte-for-byte identical to the schedule
            # the --max-sem-num flag demonstrably helps: y monolithic then
            # 4 per-group x DMAs per batch, all on the sync queue.
            x_tiles, y_tiles = [], []
            for b in range(BL):
                x_t = iopool.tile([P, NH, CPB, D], F32, tag="x", name=f"x{b}")
                y_t = iopool.tile([P, NH * CPB * D], F32, tag="y", name=f"y{b}")
                x_tiles.append(x_t)
                y_tiles.append(y_t)
                nc.sync.dma_start(
                    y_t[:], y_d[b].rearrange("d (nh n) -> (d nh) n", nh=NH)
                )
                xsrc = x_d[b].rearrange("(nh q c) d -> q nh c d", nh=NH, c=CPB)
                for g in range(NG):
                    nc.sync.dma_start(
                        x_t[:, :, g * GC : (g + 1) * GC, :],
                        xsrc[:, :, g * GC : (g + 1) * GC, :],
                    )

            # Software-pipelined compute: DVE is in-order, so emitting
            # sub(i) -> reduce(i) back-to-back idles DVE ~1us per chunk
            # waiting on ACT's square.  Emit sub(i+1) BEFORE reduce(i)
            # instead.  The last batch's chunks taper (8,8,8,4,4 c-cols)
            # so the post-stream chain is short, and sqrt runs per batch
            # (hidden under the stream for batches 0-2).
            CHUNKS = [(c, c + GC) for c in range(0, CPB, GC)]
            CHUNKS_LAST = [(0, 8), (8, 16), (16, 24), (24, 28), (28, 32)]
            pda = rpool.tile([P, BL, NG, NH, GC], F32, tag="pd")
            yT = {}

            def emit_transposes(b):
                y_v = y_tiles[b][:].rearrange("p (q c) -> p c q", c=CPB)
                for g in range(NG):
                    t = ppool.tile([P, GC, P], F32, tag="yT", name=f"yT{b}_{g}")
                    for c in range(GC):
                        nc.tensor.transpose(
                            t[:, c, :], y_v[:, g * GC + c, :], ident[:]
                        )
                    yT[(b, g)] = t

            def emit_sub_sq(b, c0, c1):
                g, gc0 = c0 // GC, c0 % GC
                w = c1 - c0
                diff = wpool.tile([P, NH, w, D], F32, tag="diff", name=f"df{b}{c0}")
                nc.vector.tensor_sub(
                    diff[:],
                    x_tiles[b][:, :, c0:c1, :],
                    yT[(b, g)][:, gc0 : gc0 + w, :].rearrange(
                        "p c (d nh) -> p nh c d", nh=NH
                    ),
                )
                sq = wpool.tile([P, NH, w, D], F32, tag="sq", name=f"sq{b}{c0}")
                nc.scalar.activation(
                    sq[:], diff[:], mybir.ActivationFunctionType.Square
                )
                return sq

            def emit_reduce(b, c0, c1, sq):
                g, gc0 = c0 // GC, c0 % GC
                nc.vector.tensor_reduce(
                    d2a[:, b, g, :, gc0 : gc0 + (c1 - c0)],
                    sq[:],
                    axis=mybir.AxisListType.X,
                    op=mybir.AluOpType.add,
                )
                if c1 == CPB:
                    nc.scalar.activation(
                        pda[:, b], d2a[:, b], mybir.ActivationFunctionType.Sqrt
                    )

            all_chunks = []
            for b in range(BL):
                for c0, c1 in (CHUNKS_LAST if b == BL - 1 else CHUNKS):
                    all_chunks.append((b, c0, c1))

            pending = None
            for b, c0, c1 in all_chunks:
                if c0 == 0:
                    emit_transposes(b)
                sq = emit_sub_sq(b, c0, c1)
                if pending is not None:
                    emit_reduce(*pending)
                pending = (b, c0, c1, sq)
            emit_reduce(*pending)

            # One contiguous store for all batches (per-batch strided
            # stores interleave small descriptors into the input stream).
            nc.sync.dma_start(o_d[:], pda[:])
    nc.finalize()
    return nc


_NC_CACHE: list = []


def _get_program() -> bass.Bass:
    if not _NC_CACHE:
        _NC_CACHE.append(_build())
    return _NC_CACHE[0]


def kernel(x: np.ndarray, y: np.ndarray) -> np.ndarray:
    x = np.ascontiguousarray(np.asarray(x, dtype=np.float32))
    y = np.ascontiguousarray(np.asarray(y, dtype=np.float32))
    assert x.shape == (B, N, D) and y.shape == (B, D, N)

    nc = _get_program()
    in_maps = [
        {"x": x[i * BL : (i + 1) * BL], "y": y[i * BL : (i + 1) * BL]}
        for i in range(NCORES)
    ]
    res = run_bass_kernel_spmd(nc, in_maps, list(range(NCORES)))
    o = np.stack([res.results[i]["o"] for i in range(NCORES)])  # [8, P, BL, NG, NH, GC]
    # o[core, p, b, g, nh, c] = pd[core*BL + b, nh*4096 + p*32 + g*8 + c]
    pd = (
        o.transpose(0, 2, 4, 1, 3, 5)  # (core, b, nh, p, g, c)
        .reshape(B, N)
    )

    dist = pd.mean(axis=0, dtype=np.float64)
    dist[1:3] *= 1.5
    return np.asarray(dist.mean(), dtype=np.float32)
